# revision 29
# baseline (speedup 1.0000x reference)
"""Trainium2 Bass kernel for NeuralTensorLayer (order-1/2/3 polynomial layer).

    out[b,l] = bias[l] + sum_i X[b,i] W1[i,l]
             + sum_ij X[b,i] X[b,j] W2[i,j,l]
             + sum_ijk X[b,i] X[b,j] X[b,k] W3[i,j,k,l]

B=32768, D=K=32, data-parallel over 8 NeuronCores (4096 rows each).

Strategy:
  * Full (i,j,k) symmetrization: out3 = sum_{a<=b} Z_ab sum_{c>=b} X_c
    W3f[(a,b),c,l] with W3f summing all distinct permutations of the
    sorted triple.  Pairs sorted by b make W3f's k-support a suffix
    [j0_chunk, 32) per 128-pair chunk (widths [32,17,10,5,1]): only the
    5984 unique triples are streamed instead of 16896 MACs.
  * k-split at c*=8: triples whose third factor index is < 8
    (120 (pair,k) combos) are precomputed on host as triple products
    Y[b,(p,k)] = Z_p X_k and contracted in the 32-col "low" matmul, so
    the big PSUM grid is [32 l x 24 k] (cols k=8..31, l-major, holes in
    a [128,1024] 2-bank tile).  This shrinks every stage-2 op by 25%.
  * Chunk 4's 16 pairs (j=31) contribute k=31 only - two 16-col matmuls
    straight into the big grid.
  * Z pair products and Y triples are precomputed on host (fp16) and
    DMA'd in - no on-device pair building.
  * Matmul operands fp16 (10-bit mantissa); stage-2 tensors bf16 (the
    DVE 2x packed mode exists for bf16 only).
  * Stage 2, two tiles per batch: ScalarE compacts PSUM (l, k>=8) ->
    [128,768] bf16; DVE does the X_k broadcast multiply, 24->12->6
    fold adds, a 6-wide reduce, and adds the pair's low PSUM bank
    (one accumulation group spans both tiles' disjoint 32-col halves)
    into the fp32 output slice.  GpSimd stays off SBUF during compute
    (its read port is shared with DVE).
  * DMA: Z ships as one [128,4096] block per supertile plus one merged
    AUX block (chunk-4 rows | Y rows | X bits); weights as one tensor.
    dma_start costs ~0.7us of issue time on the queueing engine, so
    transfers are few, big, and strictly need-ordered to avoid racing
    the 358 GB/s aggregate DMA bandwidth at startup.
  * 9 warm-up matmuls keep the PE busy until the first data lands so
    the HAM clock-gate reaches 2.4 GHz by ~11us and stays there.
  * PSUM: big [128,1024] x3 + pair-low [128,64] x2 = 8 banks.
"""

import numpy as np
import ml_dtypes
from contextlib import ExitStack

import concourse.bass as bass
import concourse.bacc as bacc
import concourse.tile as tile
from concourse import mybir
from concourse import bass_utils

FP16 = np.float16
BF16 = ml_dtypes.bfloat16

B, D, KOUT = 32768, 32, 32
NCORES = 8
BLOC = B // NCORES          # 4096 rows per core
P = 128                     # rows per tile
SUPER = 8                   # tiles per supertile
SP = SUPER * P              # 1024
NSUPER = BLOC // SP         # 4
NDUMMY = 9                  # PE warm-up matmuls (HAM un-throttle)
CSTAR = 8                   # k-columns below this go through Y-expansion
GRID = D - CSTAR            # 24 k-columns in the big grid
NV = GRID // 2 + 2          # v width per l: 12 folds + low + pad

# pairs (i,j), i<=j, sorted by j then i: p = j(j+1)/2 + i
PAIRS = [(i, j) for j in range(D) for i in range(j + 1)]
NPAIRS = len(PAIRS)         # 528
I_P = np.array([p[0] for p in PAIRS], np.int64)
J_P = np.array([p[1] for p in PAIRS], np.int64)
J0 = [int(J_P[128 * c]) for c in range(4)]      # [0, 15, 22, 27]
W_C = [D - max(j, CSTAR) for j in J0]           # big widths [24,17,10,5]
JW = [max(j, CSTAR) for j in J0]                # big window starts [8,15,22,27]

# Y-expansion rows: (pair, k) for k < CSTAR, j(pair) <= k, sorted by (k, p)
YROWS = [(p, k) for k in range(CSTAR) for p in range(NPAIRS) if J_P[p] <= k]
NY = len(YROWS)             # 120
NY_A = 128 - 16 - D         # 80 Y rows in chunk 4a
NY_B = NY - NY_A            # 40 Y rows in chunk 4b

WOFF = []
_o = 0
for _c in range(4):
    WOFF.append(_o)
    _o += 32 * W_C[_c] + 32
WOFF.append(_o)          # w4a at WOFF[4], 64 cols
_o += 64
WOFF.append(_o)          # w4b at WOFF[5], 32 cols
WCOLS = _o + 32          # 2016
AUXW = SP + SUPER * GRID        # z4a | xbd bits = 1216

F32 = mybir.dt.float32
F16 = mybir.dt.float16
BF = mybir.dt.bfloat16


def _symmetrize(W1, W2, W3):
    W1 = np.asarray(W1, np.float64)
    W2 = np.asarray(W2, np.float64)
    W3 = np.asarray(W3, np.float64)
    from itertools import permutations
    S6 = np.zeros((D, D, D, KOUT))
    for perm in set(permutations((0, 1, 2))):
        S6 += np.transpose(W3, perm + (3,))
    W3f = np.zeros((NPAIRS, D, KOUT))
    for p, (a, bb) in enumerate(PAIRS):
        for c in range(bb, D):
            if a == bb == c:
                f = 1.0 / 6.0
            elif a == bb or bb == c:
                f = 0.5
            else:
                f = 1.0
            W3f[p, c] = S6[a, bb, c] * f
    W2s = np.empty((NPAIRS, KOUT))
    for p, (a, bb) in enumerate(PAIRS):
        W2s[p] = W2[a, bb] + W2[bb, a] if a < bb else W2[a, a]
    return W1, W2s, W3f


def _pack_weights(W1, W2, W3):
    """w0..w3: [128, 32*w + 32] fp16; w4a: [128, 64]; w4b: [40, 32]."""
    W1, W2s, W3f = _symmetrize(W1, W2, W3)
    ws = []
    for c in range(4):
        j0, w = JW[c], W_C[c]
        wt = np.zeros((128, 32 * w + 32))
        blk = W3f[128 * c:128 * (c + 1), j0:, :]        # [128, w(k), 32(l)]
        blk = np.transpose(blk, (0, 2, 1)).reshape(128, KOUT * w)  # (l,k)
        wt[:, :16 * w] = blk[:, :16 * w]
        wt[:, 16 * w:32 * w] = blk[:, 16 * w:]
        wt[:, 32 * w:] = W2s[128 * c:128 * (c + 1)]
        ws.append(wt.astype(np.float32).astype(FP16))
    # chunk 4a: 16 pairs + 32 W1 rows + 80 Y rows; low cols 0:32, k31 cols 32:64
    w4a = np.zeros((128, 64))
    w4a[:16, :KOUT] = W2s[512:]
    w4a[16:48, :KOUT] = W1
    for r, (p, k) in enumerate(YROWS[:NY_A]):
        w4a[48 + r, :KOUT] = W3f[p, k, :]
    w4a[:16, KOUT:] = W3f[512:, 31, :]
    ws.append(w4a.astype(np.float32).astype(FP16))
    w4b = np.zeros((NY_B, KOUT))
    for r, (p, k) in enumerate(YROWS[NY_A:]):
        w4b[r] = W3f[p, k, :]
    ws.append(w4b.astype(np.float32).astype(FP16))
    # concatenate into one [128, 2016] tensor
    wcat = np.zeros((128, WCOLS), FP16)
    off = 0
    for c in range(4):
        wcat[:, off:off + ws[c].shape[1]] = ws[c]
        off += ws[c].shape[1]
    wcat[:, off:off + 64] = ws[4]
    off += 64
    wcat[:NY_B, off:off + KOUT] = ws[5]
    return wcat


def _build_module():
    nc = bacc.Bacc("TRN2", target_bir_lowering=False, debug=False,
                   enable_asserts=False)
    ZZd = nc.dram_tensor("ZZ", [NSUPER, 128, 8 * 512], F16, kind="ExternalInput").ap()
    AUXd = nc.dram_tensor("AUX", [NSUPER, 128, AUXW], F16, kind="ExternalInput").ap()
    AUXBd = nc.dram_tensor("AUXB", [NSUPER, NY_B, SP], F16, kind="ExternalInput").ap()
    WCd = nc.dram_tensor("WC", [128, WCOLS], F16, kind="ExternalInput").ap()
    OUTd = nc.dram_tensor("OUT", [NSUPER, 128, SUPER * KOUT], F32, kind="ExternalOutput").ap()

    with ExitStack() as ctx:
        tc = ctx.enter_context(tile.TileContext(nc))
        consts = ctx.enter_context(tc.tile_pool(name="consts", bufs=1))
        zzpool = ctx.enter_context(tc.tile_pool(name="zzpool", bufs=2))
        z4pool = ctx.enter_context(tc.tile_pool(name="z4pool", bufs=2))
        xbpool = ctx.enter_context(tc.tile_pool(name="xbpool", bufs=2))
        spool = ctx.enter_context(tc.tile_pool(name="spool", bufs=4))
        upool = ctx.enter_context(tc.tile_pool(name="upool", bufs=4))
        vpool = ctx.enter_context(tc.tile_pool(name="vpool", bufs=4))
        opool = ctx.enter_context(tc.tile_pool(name="opool", bufs=2))
        bigps = ctx.enter_context(tc.tile_pool(name="bigps", bufs=3, space="PSUM"))
        lowps = ctx.enter_context(tc.tile_pool(name="lowps", bufs=2, space="PSUM"))

        g = consts.tile([128, 640], F16, tag="g")
        nc.vector.memset(g, 0.0)

        wcat = consts.tile([128, WCOLS], F16, tag="wcat")
        w_sb = [wcat[:, WOFF[c]:WOFF[c] + 32 * W_C[c] + 32] for c in range(4)]
        w4a_sb = wcat[:, WOFF[4]:WOFF[4] + 64]
        w4b_sb = wcat[0:NY_B, WOFF[5]:WOFF[5] + KOUT]

        zz = {}
        aux = {}

        def fetch_super(s, eng, part=None):
            if part in (None, 0):
                zt = zzpool.tile([128, 8 * 512], F16, tag="zz", name=f"zz{s}")
                zz[s] = zt
            if part is None:
                eng.dma_start(out=zz[s], in_=ZZd[s])
            elif part == 0:
                eng.dma_start(out=zz[s][:, 0:2048], in_=ZZd[s][:, 0:2048])
            else:
                eng.dma_start(out=zz[s][:, 2048:4096], in_=ZZd[s][:, 2048:4096])

        def fetch_aux(s, eng):
            at = z4pool.tile([128, AUXW], F16, tag="aux", name=f"aux_{s}")
            bt = z4pool.tile([NY_B, SP], F16, tag="auxb", name=f"auxb_{s}")
            aux[s] = (at, bt)
            eng.dma_start(out=at, in_=AUXd[s])
            eng.dma_start(out=bt, in_=AUXBd[s])

        # ---- startup DMAs (need-ordered; avoid racing the DMA bandwidth)
        zt0 = zzpool.tile([128, 8 * 512], F16, tag="zz", name="zz0")
        zz[0] = zt0
        nc.sync.dma_start(out=zt0[:, 0:1024], in_=ZZd[0][:, 0:1024])
        nc.scalar.dma_start(out=wcat[:, 0:WOFF[1]], in_=WCd[:, 0:WOFF[1]])
        nc.scalar.dma_start(out=wcat[:, WOFF[1]:WCOLS], in_=WCd[:, WOFF[1]:WCOLS])
        fetch_aux(0, nc.gpsimd)
        nc.sync.dma_start(out=zt0[:, 1024:2048], in_=ZZd[0][:, 1024:2048])
        nc.sync.dma_start(out=zt0[:, 2048:4096], in_=ZZd[0][:, 2048:4096])

        # PE warm-up (results discarded; tiles recycled by the pool),
        # then dependency-free weight loads to bridge the early data-wait
        # holes so the HAM activity window never resets
        for _ in range(NDUMMY):
            dummy = bigps.tile([128, 1024], F32, tag="big")
            nc.tensor.matmul(dummy[:, 0:512], g[:, :128], g[:, 128:640],
                             start=True, stop=True)
        for _ in range(20):
            nc.tensor.ldweights(g[:, 0:128])

        for s in range(NSUPER):
            osb = opool.tile([128, SUPER * KOUT], F32, tag="osb")
            for t in range(SUPER):
                big = bigps.tile([128, 1024], F32, tag="big")
                half = t % 2
                if half == 0:
                    low2 = lowps.tile([128, 64], F32, tag="low", name=f"low_{s}_{t}")
                    _low2 = low2
                else:
                    low2 = _low2
                low = low2[:, half * 32:half * 32 + 32]
                bigv = big[:, :].rearrange("p (l k) -> p l k", k=D)
                # chunks 0-3: suffix k windows, strided PSUM writes
                for c in range(4):
                    j0, w = JW[c], W_C[c]
                    zc = zz[s][:, t * 512 + c * 128: t * 512 + (c + 1) * 128]
                    first = c == 0
                    nc.tensor.matmul(bigv[:, 0:16, j0:D], zc, w_sb[c][:, 0:16 * w],
                                     start=first, stop=False)
                    nc.tensor.matmul(bigv[:, 16:32, j0:D], zc, w_sb[c][:, 16 * w:32 * w],
                                     start=first, stop=False)
                    nc.tensor.matmul(low, zc, w_sb[c][:, 32 * w:32 * w + 32],
                                     start=first and half == 0, stop=False)
                # chunk 4a: 16 pairs (k=31 into big) + W1 rows + 80 Y rows (low)
                za = aux[s][0][:, t * 128:(t + 1) * 128]
                nc.tensor.matmul(bigv[:, 0:16, 31:32], za[0:16, :],
                                 w4a_sb[0:16, 32:48], start=False, stop=True)
                nc.tensor.matmul(bigv[:, 16:32, 31:32], za[0:16, :],
                                 w4a_sb[0:16, 48:64], start=False, stop=True)
                nc.tensor.matmul(low, za, w4a_sb[:, 0:32],
                                 start=False, stop=False)
                # chunk 4b: 40 more Y rows (low)
                zb = aux[s][1][:, t * 128:(t + 1) * 128]
                nc.tensor.matmul(low, zb, w4b_sb,
                                 start=False, stop=half == 1)

                # ---- stage 2 (2-tile batched, all DVE after the scalar copy)
                if half == 0:
                    ustage = upool.tile([128, 2 * KOUT * GRID], BF, tag="u",
                                        name=f"u_{s}_{t}")
                    vt = vpool.tile([128, 2 * KOUT * NV], BF, tag="v",
                                    name=f"v_{s}_{t}")
                    _cache2 = (ustage, vt)
                else:
                    ustage, vt = _cache2
                staged = spool.tile([128, KOUT * GRID], BF, tag="staged",
                                    name=f"staged_{s}_{t}")
                st3 = staged[:, :].rearrange("p (l k) -> p l k", k=GRID)
                nc.scalar.copy(out=st3, in_=bigv[:, :, CSTAR:D])
                u3t = (ustage[:, half * KOUT * GRID:(half + 1) * KOUT * GRID]
                       .rearrange("p (l k) -> p l k", k=GRID))
                xk = (aux[s][0][:, SP + t * GRID:SP + (t + 1) * GRID]
                      .bitcast(BF)
                      .unsqueeze(1).broadcast_to([P, KOUT, GRID]))
                nc.vector.tensor_mul(u3t, st3, xk)
                if half == 1:
                    u3 = ustage[:, :].rearrange("p (l k) -> p l k", k=GRID)
                    v3 = vt[:, :].rearrange("p (l c) -> p l c", c=NV)
                    # fold 24 -> 12, then 12 -> 6 (in place); low added from PSUM
                    nc.vector.tensor_add(v3[:, :, 0:12],
                                         u3[:, :, 0:12], u3[:, :, 12:24])
                    nc.vector.tensor_add(v3[:, :, 6:12],
                                         v3[:, :, 0:6], v3[:, :, 6:12])
                    oslice = osb[:, (t - 1) * KOUT:(t + 1) * KOUT]
                    nc.vector.reduce_sum(
                        out=oslice,
                        in_=v3[:, :, 6:12],
                        axis=mybir.AxisListType.X)
                    nc.vector.tensor_add(oslice, oslice, low2)

                # ---- prefetch next supertile
                if s + 1 < NSUPER:
                    if t == 0:
                        fetch_super(s + 1, nc.sync, part=0)
                    if t == 1:
                        fetch_aux(s + 1, nc.sync)
                    if t == 2:
                        fetch_super(s + 1, nc.sync, part=1)
            nc.sync.dma_start(out=OUTd[s], in_=osb)
            if s == 0:
                # mask the startup-bandwidth stall before supertile 1 with
                # dependency-free weight loads that keep the PE array active
                # (no PSUM write, so no coupling to stage-2 evacuations)
                for _ in range(32):
                    nc.tensor.ldweights(g[:, 0:128])
    nc.compile()
    return nc


_CACHE = {}


def _get_module():
    if "nc" not in _CACHE:
        _CACHE["nc"] = _build_module()
    return _CACHE["nc"]


def _host_inputs(X, W1, W2, W3):
    Xf = np.asarray(X, np.float32)
    Xh = Xf.astype(FP16)
    Xhf = Xh.astype(np.float32)
    Z = (Xhf[:, I_P] * Xhf[:, J_P]).astype(FP16)        # [B, 528]
    YP = np.array([r[0] for r in YROWS])
    YK = np.array([r[1] for r in YROWS])
    Y = (Z[:, YP].astype(np.float32) * Xhf[:, YK]).astype(FP16)  # [B, 120]
    Xb = Xf.astype(BF16)
    ws = _pack_weights(W1, W2, W3)

    in_maps = []
    for core in range(NCORES):
        lo, hi = core * BLOC, (core + 1) * BLOC
        view = Z[lo:hi].reshape(NSUPER, SUPER, P, NPAIRS)     # [s,t,r,p]
        zz = (view[:, :, :, :512].reshape(NSUPER, SUPER, P, 4, 128)
              .transpose(0, 4, 1, 3, 2)                       # [s, p, t, c, r]
              .reshape(NSUPER, 128, 4096))
        yv = Y[lo:hi].reshape(NSUPER, SUPER, P, NY)           # [s,t,r,y]
        auxm = np.zeros((NSUPER, 128, AUXW), FP16)
        auxm[:, 0:16, :SP] = view[:, :, :, 512:].transpose(0, 3, 1, 2).reshape(NSUPER, 16, SP)
        auxm[:, 16:48, :SP] = (Xh[lo:hi].reshape(NSUPER, SUPER, P, D)
                               .transpose(0, 3, 1, 2).reshape(NSUPER, D, SP))
        auxm[:, 48:128, :SP] = (yv[:, :, :, :NY_A]
                                .transpose(0, 3, 1, 2).reshape(NSUPER, NY_A, SP))
        auxb = np.ascontiguousarray(yv[:, :, :, NY_A:]
                                    .transpose(0, 3, 1, 2).reshape(NSUPER, NY_B, SP))
        xbd = (Xb[lo:hi, CSTAR:].reshape(NSUPER, SUPER, P, GRID)
               .transpose(0, 2, 1, 3).reshape(NSUPER, 128, SUPER * GRID))
        auxm[:, :, SP:] = np.ascontiguousarray(xbd).view(np.float16)
        m = {
            "ZZ": np.ascontiguousarray(zz),
            "AUX": auxm,
            "AUXB": auxb,
            "WC": ws,
        }
        in_maps.append(m)
    return in_maps


def kernel(X, W1, W2, W3, bias):
    bias = np.asarray(bias, np.float32)
    in_maps = _host_inputs(X, W1, W2, W3)
    nc = _get_module()
    res = bass_utils.run_bass_kernel_spmd(nc, in_maps, core_ids=list(range(NCORES)))
    _CACHE["last_results"] = res
    outs = []
    for c in range(NCORES):
        od = np.asarray(res.results[c]["OUT"])       # [NSUPER, 128, SUPER*KOUT]
        outs.append(od.reshape(NSUPER, P, SUPER, KOUT)
                    .transpose(0, 2, 1, 3).reshape(BLOC, KOUT))
    out = np.concatenate(outs, 0)
    return (out + bias.reshape(1, KOUT)).astype(np.float32)
